# revision 27
# baseline (speedup 1.0000x reference)
import sys

sys.path.insert(0, "/opt/trn_rl_repo")

import numpy as np

import concourse.bass as bass
import concourse.bacc as bacc
import concourse.mybir as mybir
import concourse.tile as tile
from concourse import bass_utils

BF16 = mybir.dt.bfloat16
F32 = mybir.dt.float32
AF = mybir.ActivationFunctionType

B, L, D = 2, 2048, 1024
H, HD = 16, 64
G = 4
GD = D // G
SCALE = HD ** -0.5
NKT = D // 128
NLK = L // 128
P = 128


def _build(dbg=False, repeat=1):
    nc = bacc.Bacc("TRN2", target_bir_lowering=False, debug=False, num_devices=8)

    xqT = nc.dram_tensor("xqT", [D, L], BF16, kind="ExternalInput")
    xkT = nc.dram_tensor("xkT", [D, L], BF16, kind="ExternalInput")
    xvT = nc.dram_tensor("xvT", [D, L], BF16, kind="ExternalInput")
    wqT = nc.dram_tensor("wqT", [D, GD], BF16, kind="ExternalInput")
    wkT = nc.dram_tensor("wkT", [D, GD], BF16, kind="ExternalInput")
    wvT = nc.dram_tensor("wvT", [D, GD], BF16, kind="ExternalInput")
    woT = nc.dram_tensor("woT", [GD, D], BF16, kind="ExternalInput")
    bqv = nc.dram_tensor("bqv", [P, GD // P], F32, kind="ExternalInput")
    bkv = nc.dram_tensor("bkv", [P, GD // P], F32, kind="ExternalInput")
    bvv = nc.dram_tensor("bvv", [1, GD], F32, kind="ExternalInput")
    maskb = nc.dram_tensor("maskb", [P, NLK], F32, kind="ExternalInput")
    vones = nc.dram_tensor("vones", [NLK, G], BF16, kind="ExternalInput")
    ones64 = nc.dram_tensor("ones64", [1, HD], BF16, kind="ExternalInput")
    out = nc.dram_tensor("out", [L, D], BF16, kind="ExternalOutput")

    with tile.TileContext(nc) as tc, \
            nc.allow_low_precision(reason="bf16 matmul pipeline, fp32 accumulation in PSUM"):
        with tc.tile_pool(name="wp", bufs=1) as wp, \
                tc.tile_pool(name="cn", bufs=1) as cn, \
                tc.tile_pool(name="xp", bufs=2) as xp, \
                tc.tile_pool(name="big", bufs=1) as big, \
                tc.tile_pool(name="tt", bufs=3) as ttp, \
                tc.tile_pool(name="sm", bufs=2) as smp, \
                tc.tile_pool(name="ob", bufs=2) as obp, \
                tc.tile_pool(name="ps", bufs=3, space="PSUM") as psp, \
                tc.tile_pool(name="po", bufs=2, space="PSUM") as pop:

            rep_cm = tc.For_i(0, repeat, 1) if repeat > 1 else None
            if rep_cm is not None:
                rep_cm.__enter__()

            wq_t = wp.tile([P, NKT, GD], BF16, tag="wq")
            wk_t = wp.tile([P, NKT, GD], BF16, tag="wk")
            wv_t = wp.tile([P, NKT, GD], BF16, tag="wv")
            wo_t = wp.tile([P, GD // P, D], BF16, tag="wo")
            for w_t, w_d in ((wq_t, wqT), (wk_t, wkT), (wv_t, wvT)):
                wr = w_d.ap().rearrange("(kt p) m -> p kt m", p=P)
                for kh in range(2):
                    nc.sync.dma_start(out=w_t[:, kh * 4:(kh + 1) * 4, :],
                                      in_=wr[:, kh * 4:(kh + 1) * 4, :])
            wor = woT.ap().rearrange("(kt p) m -> p kt m", p=P)
            for kt in range(2):
                nc.sync.dma_start(out=wo_t[:, kt, :], in_=wor[:, kt, :])

            bq_t = cn.tile([P, GD // P], F32, tag="bq")
            bk_t = cn.tile([P, GD // P], F32, tag="bk")
            nc.sync.dma_start(out=bq_t, in_=bqv.ap())
            nc.sync.dma_start(out=bk_t, in_=bkv.ap())
            bvb_t = cn.tile([P, G, HD], F32, tag="bvb")
            _bv = bvv.ap()
            nc.sync.dma_start(
                out=bvb_t,
                in_=bass.AP(tensor=_bv.tensor, offset=_bv.offset,
                            ap=[[0, P], [HD, G], [1, HD]]))
            mb_t = cn.tile([P, NLK], F32, tag="mb")
            nc.sync.dma_start(out=mb_t, in_=maskb.ap())
            ones64_t = cn.tile([1, HD], BF16, tag="ones64")
            nc.sync.dma_start(out=ones64_t, in_=ones64.ap())

            qt_t = [[big.tile([P, L // 2], BF16, tag=f"qt{mt}{hf}",
                               name=f"qt{mt}{hf}")
                     for hf in range(2)] for mt in range(2)]
            ot_t = [[big.tile([P, L // 2], BF16, tag=f"ot{mt}{hf}",
                              name=f"ot{mt}{hf}")
                     for hf in range(2)] for mt in range(2)]
            kp_t = big.tile([P, G, NLK, P], BF16, tag="kp")
            for h in range(G):
                nc.vector.memset(kp_t[:, h, :, :], 0.0)
            va_t = [big.tile([P, NLK // 2, G, HD + 1], BF16, tag=f"va{hf}",
                             name=f"va{hf}")
                    for hf in range(2)]
            _vo = vones.ap()
            vo_stage = cn.tile([P, NLK * G], BF16, tag="vost")
            nc.sync.dma_start(
                out=vo_stage,
                in_=bass.AP(tensor=_vo.tensor, offset=_vo.offset,
                            ap=[[0, P], [1, NLK * G]]))
            for hf in range(2):
                nc.vector.tensor_copy(
                    va_t[hf][:, :, :, HD:HD + 1],
                    vo_stage[:, hf * NLK * G // 2:(hf + 1) * NLK * G // 2]
                    .rearrange("p (a b one) -> p a b one", a=NLK // 2, b=G))

            for half in range(2):
                x_t = xp.tile([P, NKT, L // 2], BF16, tag="x")
                xr = xkT.ap().rearrange("(kt p) n -> p kt n", p=P)
                for kt in range(NKT):
                    nc.sync.dma_start(
                        out=x_t[:, kt, :],
                        in_=xr[:, kt, half * (L // 2):(half + 1) * (L // 2)])
                for mt in range(2):
                    for ch in range(2):
                        ps = psp.tile([P, 512], F32, tag="ps")
                        for kt in range(NKT):
                            nc.tensor.matmul(
                                ps[:],
                                wk_t[:, kt, mt * P:(mt + 1) * P],
                                x_t[:, kt, ch * 512:(ch + 1) * 512],
                                start=(kt == 0), stop=(kt == NKT - 1))
                        lk0 = half * 8 + ch * 4
                        for sub in range(2):
                            h = 2 * mt + sub
                            po = sub * HD
                            nc.vector.tensor_scalar_add(
                                kp_t[po:po + HD, h, lk0:lk0 + 4, :]
                                .rearrange("p a b -> p (a b)"),
                                ps[po:po + HD, :],
                                bk_t[po:po + HD, mt:mt + 1])

            for half in range(2):
                x_t = xp.tile([P, NKT, L // 2], BF16, tag="x")
                xr = xqT.ap().rearrange("(kt p) n -> p kt n", p=P)
                for kt in range(NKT):
                    nc.sync.dma_start(
                        out=x_t[:, kt, :],
                        in_=xr[:, kt, half * (L // 2):(half + 1) * (L // 2)])
                for mt in range(2):
                    for ch in range(2):
                        ps = psp.tile([P, 512], F32, tag="ps")
                        for kt in range(NKT):
                            nc.tensor.matmul(
                                ps[:],
                                wq_t[:, kt, mt * P:(mt + 1) * P],
                                x_t[:, kt, ch * 512:(ch + 1) * 512],
                                start=(kt == 0), stop=(kt == NKT - 1))
                        nc.vector.tensor_scalar_add(
                            qt_t[mt][half][:, ch * 512:(ch + 1) * 512], ps[:],
                            bq_t[:, mt:mt + 1])

            for half in range(2):
                x_t = xp.tile([P, NKT, L // 2], BF16, tag="x")
                xr = xvT.ap().rearrange("(kt p) n -> p kt n", p=P)
                for kt in range(NKT):
                    nc.sync.dma_start(
                        out=x_t[:, kt, :],
                        in_=xr[:, kt, half * (L // 2):(half + 1) * (L // 2)])
                for loc in range(NLK // 2):
                    ps = psp.tile([P, GD], F32, tag="ps")
                    for kt in range(NKT):
                        nc.tensor.matmul(
                            ps[:], x_t[:, kt, loc * P:(loc + 1) * P], wv_t[:, kt, :],
                            start=(kt == 0), stop=(kt == NKT - 1))
                    nc.vector.tensor_tensor(
                        out=va_t[half][:, loc, :, 0:HD],
                        in0=ps[:].rearrange("p (h d) -> p h d", h=G),
                        in1=bvb_t[:],
                        op=mybir.AluOpType.add)

            def attn_head(ch, h):
                mt, po = h // 2, (h % 2) * HD
                ps_o = [pop.tile([HD + 1, 512], F32, tag="po",
                                 name=f"po{ch}{h}{sc}") for sc in range(2)]

                def s_mm(lk):
                    ps_s = psp.tile([P, L // 2], F32, tag="ps")
                    for sc in range(2):
                        nc.tensor.matmul(
                            ps_s[:, sc * 512:(sc + 1) * 512],
                            kp_t[:, h, lk, :],
                            qt_t[mt][ch][:, sc * 512:(sc + 1) * 512],
                            start=True, stop=True)
                    return ps_s

                ps_s_cur = s_mm(0)
                for lk in range(NLK):
                    tt = ttp.tile([P, L // 2], BF16, tag="tt")
                    nc.scalar.activation(tt[:], ps_s_cur[:], AF.Exp,
                                         bias=mb_t[:, lk:lk + 1], scale=SCALE)
                    if lk + 1 < NLK:
                        ps_s_cur = s_mm(lk + 1)
                    for sc in range(2):
                        nc.tensor.matmul(
                            ps_o[sc][:],
                            va_t[lk // 8][:, lk % 8, h, :],
                            tt[:, sc * 512:(sc + 1) * 512],
                            start=(lk == 0), stop=(lk == NLK - 1))
                for sc in range(2):
                    den0 = smp.tile([1, 512], F32, tag="den0")
                    nc.vector.tensor_copy(den0[:], ps_o[sc][HD:HD + 1, :])
                    rec = smp.tile([1, 512], F32, tag="rec")
                    nc.vector.reciprocal_approx_fast(rec[:], den0[:])
                    recb = smp.tile([HD, 512], F32, tag="recb")
                    nc.gpsimd.partition_broadcast(recb[:], rec[:])
                    nc.vector.tensor_mul(
                        ot_t[mt][ch][po:po + HD, sc * 512:(sc + 1) * 512],
                        ps_o[sc][0:HD, :], recb[:])

            def outproj(ch):
                for qt in range(8):
                    qa = ch * 8 + qt
                    ob = obp.tile([P, D], BF16, tag="ob")
                    for nch in range(2):
                        ps_u = psp.tile([P, 512], F32, tag="ps")
                        for kt in range(2):
                            nc.tensor.matmul(
                                ps_u[:],
                                ot_t[kt][ch][:, qt * P:(qt + 1) * P],
                                wo_t[:, kt, nch * 512:(nch + 1) * 512],
                                start=(kt == 0), stop=(kt == 1))
                        nc.vector.tensor_copy(ob[:, nch * 512:(nch + 1) * 512], ps_u[:])
                    nc.sync.dma_start(out=out.ap()[qa * P:(qa + 1) * P, :], in_=ob[:])

            for h in range(G):
                attn_head(0, h)
            attn_head(1, 0)
            outproj(0)
            for h in range(1, G):
                attn_head(1, h)
            outproj(1)

            if rep_cm is not None:
                rep_cm.__exit__(None, None, None)

    nc.compile()
    return nc


_NC = None


def _get_nc():
    global _NC
    if _NC is None:
        _NC = _build()
    return _NC


def _bf16(x):
    import ml_dtypes
    return np.ascontiguousarray(x, dtype=np.float32).astype(ml_dtypes.bfloat16)


def _build_in_maps(q, k, v, kv_mask, Wq, bq, Wk, bk, Wv, bv, Wo, bo):
    q = np.asarray(q, np.float32)
    k = np.asarray(k, np.float32)
    v = np.asarray(v, np.float32)
    kv_mask = np.asarray(kv_mask)
    Wq, bq = np.asarray(Wq, np.float32), np.asarray(bq, np.float32)
    Wk, bk = np.asarray(Wk, np.float32), np.asarray(bk, np.float32)
    Wv, bv = np.asarray(Wv, np.float32), np.asarray(bv, np.float32)
    Wo = np.asarray(Wo, np.float32)

    xT = {b: {"q": _bf16(q[b].T), "k": _bf16(k[b].T), "v": _bf16(v[b].T)}
          for b in range(B)}
    mb = {b: np.ascontiguousarray(
              np.where(kv_mask[b] != 0, 0.0, -1e30).astype(np.float32)
              .reshape(NLK, P).T)
          for b in range(B)}
    vones = _bf16(np.ones((NLK, G), np.float32))
    ones64 = _bf16(np.ones((1, HD), np.float32))

    wslice = {}
    for g in range(G):
        rows = slice(g * GD, (g + 1) * GD)
        wslice[g] = {
            "wqT": _bf16(Wq[rows, :].T), "wkT": _bf16(Wk[rows, :].T),
            "wvT": _bf16(Wv[rows, :].T), "woT": _bf16(Wo[:, rows].T),
            "bqv": np.ascontiguousarray(bq[rows].reshape(GD // P, P).T),
            "bkv": np.ascontiguousarray(bk[rows].reshape(GD // P, P).T),
            "bvv": bv[rows].reshape(1, GD),
        }

    in_maps = []
    for core in range(8):
        b, g = core // G, core % G
        m = {"xqT": xT[b]["q"], "xkT": xT[b]["k"], "xvT": xT[b]["v"],
             "maskb": mb[b], "vones": vones, "ones64": ones64}
        m.update(wslice[g])
        in_maps.append(m)
    return in_maps


def kernel(q, k, v, kv_mask, Wq, bq, Wk, bk, Wv, bv, Wo, bo):
    bo = np.asarray(bo, np.float32)
    in_maps = _build_in_maps(q, k, v, kv_mask, Wq, bq, Wk, bk, Wv, bv, Wo, bo)
    nc = _get_nc()
    res = bass_utils.run_bass_kernel_spmd(nc, in_maps, core_ids=list(range(8)))

    outs = [np.asarray(r["out"]).astype(np.float32) for r in res.results]
    full = np.empty((B, L, D), np.float32)
    for b in range(B):
        acc = outs[b * G]
        for g in range(1, G):
            acc += outs[b * G + g]
        full[b] = acc + bo[None, :]
    return full
